# revision 3
# baseline (speedup 1.0000x reference)
"""Masked cross-modal attention on 8 Trainium2 NeuronCores.

Reference math (per batch b):
    q,k,v = x @ W{q,k,v}.T   (head-major channels, H=8, Dh=64)
    s     = (q @ k.T) / 8, masked_fill(mask==0, 1e-9), softmax over keys
    out   = (att @ v) @ Wout.T

Key identity used here: masked positions contribute exp(1e-9)=1 weight and
v_j value, independent of the query. So with U = unmasked keys, M = masked:
    out[t] = (sum_{j in U} e^{s_tj} v_j + sum_{j in M} v_j)
           / (sum_{j in U} e^{s_tj} + |M|)
The kernel computes attention ONLY over gathered unmasked keys (about half),
and the constant masked-sum corrections are tiny host-side vectors.

Sharding: core c -> batch c//2, head-group c%2 (4 of 8 heads). Weights are
sliced per head-group; each core emits a partial [2048,512] output (its 4
heads' contribution through Wout) and the host adds the two halves per batch.
"""

import sys

for _p in ("/opt/trn_rl_repo", "/root/.axon_site/_ro/trn_rl_repo"):
    if _p not in sys.path:
        sys.path.append(_p)

import numpy as np
import concourse.bass as bass
import concourse.mybir as mybir
import concourse.tile as tile
from concourse import bacc
from concourse.bass_utils import run_bass_kernel_spmd

F32 = mybir.dt.float32
F32R = mybir.dt.float32r
EXP = mybir.ActivationFunctionType.Exp
ADD = mybir.AluOpType.add
MULT = mybir.AluOpType.mult

B, N, DIM = 4, 2048, 512
HEADS_PER_CORE = 4
DL = HEADS_PER_CORE * 64          # 256 local head dims per core
SCALE = 64 ** -0.5
TT = N // 512                     # 4 t-tiles of 512
TC = N // 128                     # 16 t-chunks of 128


def _build(nc, s_pad):
    n_sc = s_pad // 128           # s-chunks of 128 gathered keys

    xt = nc.dram_tensor("XT", [DIM, N], F32R, kind="ExternalInput")
    xgt = nc.dram_tensor("XGT", [DIM, s_pad], F32R, kind="ExternalInput")
    ind4 = nc.dram_tensor("IND4", [s_pad, 4], F32R, kind="ExternalInput")
    wqt = nc.dram_tensor("WQT", [DIM, DL], F32R, kind="ExternalInput")
    wkt = nc.dram_tensor("WKT", [DIM, DL], F32R, kind="ExternalInput")
    wvt = nc.dram_tensor("WVT", [DIM, DL], F32R, kind="ExternalInput")
    wot = nc.dram_tensor("WOT", [DL, DIM], F32R, kind="ExternalInput")
    corr = nc.dram_tensor("CORR", [65, 4], F32, kind="ExternalInput")
    out = nc.dram_tensor("OUT", [N, DIM], F32, kind="ExternalOutput")

    with tile.TileContext(nc) as tc:
        with tc.tile_pool(name="persist", bufs=1) as pp:
            # weights: [c-chunk 128, 4 chunks * 256] ; wout: [128, 2*512]
            wq_sb = pp.tile([128, 4 * DL], F32R)
            wk_sb = pp.tile([128, 4 * DL], F32R)
            wv_sb = pp.tile([128, 4 * DL], F32R)
            wo_sb = pp.tile([128, 2 * DIM], F32R)
            corr_sb = pp.tile([65, 4], F32)
            qt_sb = pp.tile([128, 2 * N], F32R)          # [d-chunk 2][t]
            kt_sb = pp.tile([128, 2 * s_pad], F32R)      # [d-chunk 2][s]
            v_sb = pp.tile([128, n_sc * 4 * 65], F32R)   # [sc][h][65]
            att_pair0 = pp.tile([128, N], F32R)
            att_pair1 = pp.tile([128, N], F32R)
            att_pair = [att_pair0, att_pair1]

            for k in range(4):
                nc.sync.dma_start(wq_sb[:, k * DL:(k + 1) * DL], wqt.ap()[k * 128:(k + 1) * 128, :])
                nc.sync.dma_start(wk_sb[:, k * DL:(k + 1) * DL], wkt.ap()[k * 128:(k + 1) * 128, :])
                nc.sync.dma_start(wv_sb[:, k * DL:(k + 1) * DL], wvt.ap()[k * 128:(k + 1) * 128, :])
            for k in range(2):
                nc.sync.dma_start(wo_sb[:, k * DIM:(k + 1) * DIM], wot.ap()[k * 128:(k + 1) * 128, :])
            nc.sync.dma_start(corr_sb[:], corr.ap())
            # indicator columns (col 64 of each head's 65-wide V block)
            v_view = v_sb[:].rearrange("p (s h x) -> p s h x", s=n_sc, h=4)
            for sc in range(n_sc):
                nc.sync.dma_start(v_view[:, sc, :, 64], ind4.ap()[sc * 128:(sc + 1) * 128, :])

            # ---------------- Phase 1: QKV projections ----------------
            with (
                tc.tile_pool(name="xpool", bufs=4) as xp,
                tc.tile_pool(name="psproj", bufs=3, space="PSUM") as pj,
            ):
                xg_tiles = []
                for k in range(4):
                    xg = xp.tile([128, s_pad], F32R, tag="xg")
                    nc.sync.dma_start(xg[:], xgt.ap()[k * 128:(k + 1) * 128, :])
                    xg_tiles.append(xg)

                # K^T[d, s] and V[s, d] from gathered tokens
                s_tiles = [(i * 512, min(512, s_pad - i * 512)) for i in range((s_pad + 511) // 512)]
                for dc in range(2):
                    for s0, sw in s_tiles:
                        pk = pj.tile([128, 512], F32, tag="pk")
                        for k in range(4):
                            nc.tensor.matmul(
                                pk[:, :sw],
                                wk_sb[:, k * DL + dc * 128: k * DL + (dc + 1) * 128],
                                xg_tiles[k][:, s0:s0 + sw],
                                start=(k == 0), stop=(k == 3),
                            )
                        nc.vector.tensor_copy(kt_sb[:, dc * s_pad + s0: dc * s_pad + s0 + sw], pk[:, :sw])
                for sc in range(n_sc):
                    pv = pj.tile([128, 256], F32, tag="pv")
                    for k in range(4):
                        nc.tensor.matmul(
                            pv[:],
                            xg_tiles[k][:, sc * 128:(sc + 1) * 128],
                            wv_sb[:, k * DL:(k + 1) * DL],
                            start=(k == 0), stop=(k == 3),
                        )
                    nc.vector.tensor_copy(
                        v_view[:, sc, :, 0:64],
                        pv[:].rearrange("p (h x) -> p h x", h=4),
                    )

                # Q^T[d, t] over all tokens
                xt_tiles = []
                for k in range(4):
                    xf = xp.tile([128, N], F32R, tag="xf")
                    nc.sync.dma_start(xf[:], xt.ap()[k * 128:(k + 1) * 128, :])
                    xt_tiles.append(xf)
                for dc in range(2):
                    for t in range(TT):
                        pq = pj.tile([128, 512], F32, tag="pk")
                        for k in range(4):
                            nc.tensor.matmul(
                                pq[:],
                                wq_sb[:, k * DL + dc * 128: k * DL + (dc + 1) * 128],
                                xt_tiles[k][:, t * 512:(t + 1) * 512],
                                start=(k == 0), stop=(k == 3),
                            )
                        nc.vector.tensor_copy(qt_sb[:, dc * N + t * 512: dc * N + (t + 1) * 512], pq[:])

            # ------------- Phase 2: scores, exp, numer, normalize -------------
            with (
                tc.tile_pool(name="epool", bufs=3) as ep,
                tc.tile_pool(name="psS", bufs=2, space="PSUM") as psS,
                tc.tile_pool(name="psN", bufs=2, space="PSUM") as psN,
                tc.tile_pool(name="npool", bufs=3) as np_pool,
                tc.tile_pool(name="dpool", bufs=2) as dpool,
                tc.tile_pool(name="dram", bufs=4, space="DRAM") as drp,
                tc.tile_pool(name="ahpool", bufs=2) as ahp,
            ):
                den_sb = pp.tile([65, 4 * N], F32)
                for hp in range(2):
                    numer_sb = {}
                    for h in (2 * hp, 2 * hp + 1):
                        numer_sb[h] = np_pool.tile([65, N], F32, tag="numer", name=f"numer{h}")
                    for t in range(TT):
                        pn = {}
                        for h in (2 * hp, 2 * hp + 1):
                            pn[h] = psN.tile([65, 512], F32, tag="pn", name=f"pn{h}")
                        # slices: (h, sc) pairs, grouped 3 per ACT op
                        slices = [(2 * hp + (i % 2), i // 2) for i in range(2 * n_sc)]
                        for gi in range(0, len(slices), 3):
                            grp = slices[gi:gi + 3]
                            reg = psS.tile([128, 1536], F32, tag="reg")
                            e_sb = ep.tile([128, 1536], F32R, tag="e")
                            for j, (h, sc) in enumerate(grp):
                                par = (h % 2) * 64
                                nc.tensor.matmul(
                                    reg[:, j * 512:(j + 1) * 512],
                                    kt_sb[par:par + 64, hp * s_pad + sc * 128: hp * s_pad + (sc + 1) * 128],
                                    qt_sb[par:par + 64, hp * N + t * 512: hp * N + (t + 1) * 512],
                                    start=True, stop=True,
                                )
                            w = len(grp) * 512
                            nc.scalar.activation(e_sb[:, :w], reg[:, :w], EXP, scale=SCALE)
                            for j, (h, sc) in enumerate(grp):
                                nc.tensor.matmul(
                                    pn[h][:],
                                    v_sb[:, (sc * 4 + h) * 65:(sc * 4 + h + 1) * 65],
                                    e_sb[:, j * 512:(j + 1) * 512],
                                    start=(sc == 0), stop=(sc == n_sc - 1),
                                )
                        for h in (2 * hp, 2 * hp + 1):
                            nc.vector.tensor_copy(numer_sb[h][:, t * 512:(t + 1) * 512], pn[h][:])

                    for h in (2 * hp, 2 * hp + 1):
                        # denominator (+|M|), reciprocal after partition-broadcast
                        nc.vector.tensor_scalar_add(
                            den_sb[64:65, h * N:(h + 1) * N],
                            numer_sb[h][64:65, :],
                            corr_sb[64:65, h:h + 1],
                        )
                        scratch = drp.tile([N], F32, tag="scr")
                        nc.sync.dma_start(scratch[:].unsqueeze(0), den_sb[64:65, h * N:(h + 1) * N])
                        bden = dpool.tile([64, N], F32, tag="bden")
                        nc.sync.dma_start(bden[:], scratch[:].unsqueeze(0).broadcast_to([64, N]))
                        rbc = dpool.tile([64, N], F32, tag="rbc")
                        nc.vector.reciprocal_approx_fast(out=rbc[:], in_=bden[:])
                        att_h = ahp.tile([64, N], F32R, tag="att")
                        nc.vector.scalar_tensor_tensor(
                            out=att_h[:], in0=numer_sb[h][0:64, :],
                            scalar=corr_sb[0:64, h:h + 1], in1=rbc[:],
                            op0=ADD, op1=MULT,
                        )
                        par = (h % 2) * 64
                        nc.sync.dma_start(att_pair[hp][par:par + 64, :], att_h[:])

            # ---------------- Phase 3: output projection ----------------
            with (
                tc.tile_pool(name="psO", bufs=4, space="PSUM") as psO,
                tc.tile_pool(name="opool", bufs=4) as op,
            ):
                for t in range(TC):
                    po = psO.tile([128, 512], F32, tag="po")
                    for dp in range(2):
                        nc.tensor.matmul(
                            po[:],
                            att_pair[dp][:, t * 128:(t + 1) * 128],
                            wo_sb[:, dp * DIM:(dp + 1) * DIM],
                            start=(dp == 0), stop=(dp == 1),
                        )
                    o_sb = op.tile([128, 512], F32, tag="o")
                    nc.vector.tensor_copy(o_sb[:], po[:])
                    nc.sync.dma_start(out.ap()[t * 128:(t + 1) * 128, :], o_sb[:])

    nc.compile()
    return nc


def kernel(input_feature, mask, Wq, Wk, Wv, Wout):
    x = np.ascontiguousarray(np.asarray(input_feature, dtype=np.float32))
    m = np.asarray(mask)
    Wq = np.asarray(Wq, dtype=np.float32)
    Wk = np.asarray(Wk, dtype=np.float32)
    Wv = np.asarray(Wv, dtype=np.float32)
    Wout = np.asarray(Wout, dtype=np.float32)

    idxs = [np.flatnonzero(m[b]) for b in range(B)]
    s_pad = max(128, ((max(len(i) for i in idxs) + 127) // 128) * 128)

    in_maps = []
    for c in range(8):
        b, g = c // 2, c % 2
        idx = idxs[b]
        cnt = len(idx)
        xg = np.zeros((s_pad, DIM), np.float32)
        xg[:cnt] = x[b][idx]
        ind4 = np.zeros((s_pad, 4), np.float32)
        ind4[:cnt] = 1.0
        xm = x[b][m[b] == 0].sum(axis=0, dtype=np.float32)
        corr = np.zeros((65, 4), np.float32)
        for h in range(4):
            hg = g * 4 + h
            corr[0:64, h] = Wv[hg * 64:(hg + 1) * 64, :] @ xm
            corr[64, h] = np.float32(N - cnt)
        in_maps.append({
            "XT": np.ascontiguousarray(x[b].T),
            "XGT": np.ascontiguousarray(xg.T),
            "IND4": ind4,
            "WQT": np.ascontiguousarray(Wq[g * DL:(g + 1) * DL, :].T),
            "WKT": np.ascontiguousarray(Wk[g * DL:(g + 1) * DL, :].T),
            "WVT": np.ascontiguousarray(Wv[g * DL:(g + 1) * DL, :].T),
            "WOT": np.ascontiguousarray(Wout[:, g * DL:(g + 1) * DL].T),
            "CORR": corr,
        })

    nc = bacc.Bacc("TRN2", target_bir_lowering=False, debug=False, num_devices=8)
    _build(nc, s_pad)
    res = run_bass_kernel_spmd(nc, in_maps, core_ids=list(range(8)))

    out = np.empty((B, N, DIM), np.float32)
    for b in range(B):
        out[b] = res.results[2 * b]["OUT"] + res.results[2 * b + 1]["OUT"]
    return out
